# revision 5
# baseline (speedup 1.0000x reference)
"""Causal multi-head attention with (buggy-faithful) RoPE on 8 trn2 cores.

Problem: B=4, S=2048, D=1024, H=16 heads of dim 64, fp32.
Sharding: core c handles batch b=c//2 and head-group g=c%2 (8 heads).
Each core computes partial_out = attn(x_b, heads g) @ wo[rows g]; the host
sums the two partials per batch and adds the bias.

Key host-side preprocessing:
- The reference's RoPE (with its cos-overwritten-by-sin bug) reduces to
  q' = (q @ (I + R')) * sin_pattern, where R' swaps/negates half-dims.
  (I + R') is folded into wq/wk on the host, so on-device RoPE is a single
  elementwise multiply fused into the PSUM->SBUF drain of the projections.
- x is supplied transposed (xT [D, S]) so every matmul contraction dim lands
  on SBUF partitions naturally.

On-device layout (per core):
- QT/KT [512, 2048] feature-major (head pairs stacked per 128-partition chunk)
- V [2048, 520] sequence-major, 65 cols per head (64 + ones col -> softmax
  denominator accumulates for free in the P@V matmul)
- S^T [sk, sq] per head: softmax denom is a partition-dim sum, handled by the
  ones column; exp needs no max subtraction (|scores/8| < ~3).
"""

import numpy as np
import ml_dtypes

import concourse.bacc as bacc
import concourse.mybir as mybir
import concourse.tile as tile
from concourse.bass_utils import run_bass_kernel_spmd

B, S, D = 4, 2048, 1024
H = 16
AOD = 64
HL = 8            # heads per core
FL = HL * AOD     # 512 local features
NCORES = 8
NCH = D // 128    # 8 contraction chunks for projections
NSQ = 4           # sq tiles of 512
NP = FL // 128    # 4 feature chunks (head pairs)
NT16 = S // 128   # 16 seq chunks of 128

F32 = mybir.dt.float32
BF16 = mybir.dt.bfloat16
EXP = mybir.ActivationFunctionType.Exp
MUL = None  # set lazily (AluOpType)

_CACHED = {}


def _alu():
    from concourse.alu_op_type import AluOpType
    return AluOpType


def build_nc():
    if "nc" in _CACHED:
        return _CACHED["nc"]
    from contextlib import ExitStack

    nc = bacc.Bacc("TRN2", target_bir_lowering=False, debug=False,
                   num_devices=NCORES)
    d_xT = nc.dram_tensor("xT", [D, S], BF16, kind="ExternalInput").ap()
    d_wq = nc.dram_tensor("wq", [D, FL], BF16, kind="ExternalInput").ap()
    d_wk = nc.dram_tensor("wk", [D, FL], BF16, kind="ExternalInput").ap()
    d_wv = nc.dram_tensor("wv", [D, FL], BF16, kind="ExternalInput").ap()
    d_wo = nc.dram_tensor("wo", [FL, D], BF16, kind="ExternalInput").ap()
    d_sin = nc.dram_tensor("sin", [128, S], F32, kind="ExternalInput").ap()
    d_out = nc.dram_tensor("out", [S, D], F32, kind="ExternalOutput").ap()

    mult = _alu().mult

    with tile.TileContext(nc) as tc, ExitStack() as ctx:
        p_x = ctx.enter_context(tc.tile_pool(name="x", bufs=NCH))
        p_w = ctx.enter_context(tc.tile_pool(name="w", bufs=3 * NCH))
        p_wo = ctx.enter_context(tc.tile_pool(name="wo", bufs=NP))
        p_qk = ctx.enter_context(tc.tile_pool(name="qk", bufs=2 * NP))
        p_v = ctx.enter_context(tc.tile_pool(name="v", bufs=NT16))
        p_sin = ctx.enter_context(tc.tile_pool(name="sin", bufs=1))
        p_pt = ctx.enter_context(tc.tile_pool(name="pt", bufs=4))
        p_yt = ctx.enter_context(tc.tile_pool(name="yt", bufs=NP))
        p_r = ctx.enter_context(tc.tile_pool(name="r", bufs=4))
        p_os = ctx.enter_context(tc.tile_pool(name="os", bufs=4))
        ps_proj = ctx.enter_context(tc.tile_pool(name="psp", bufs=2, space="PSUM"))
        ps_s = ctx.enter_context(tc.tile_pool(name="pss", bufs=2, space="PSUM"))
        ps_o = ctx.enter_context(tc.tile_pool(name="pso", bufs=2, space="PSUM"))

        # ---- resident loads ----
        sin_sb = p_sin.tile([128, S], F32, tag="sin")
        nc.sync.dma_start(sin_sb[:], d_sin)
        x_sb = []
        for c in range(NCH):
            t = p_x.tile([128, S], BF16, tag="x")
            nc.sync.dma_start(t[:], d_xT[128 * c:128 * (c + 1), :])
            x_sb.append(t)
        wq_sb, wk_sb, wv_sb = [], [], []
        for lst, dram, nm in ((wq_sb, d_wq, "wq"), (wk_sb, d_wk, "wk"),
                              (wv_sb, d_wv, "wv")):
            for c in range(NCH):
                t = p_w.tile([128, FL], BF16, tag="w")
                nc.sync.dma_start(t[:], dram[128 * c:128 * (c + 1), :])
                lst.append(t)
        wo_sb = []
        for p in range(NP):
            t = p_wo.tile([128, D], BF16, tag="wo")
            nc.sync.dma_start(t[:], d_wo[128 * p:128 * (p + 1), :])
            wo_sb.append(t)

        # ---- V projection: V[sq, f] natural layout, 65-wide per head ----
        v_sb = []
        for q in range(NT16):
            vt = p_v.tile([128, HL, 65], BF16, tag="v")
            nc.gpsimd.memset(vt[:, :, 64:65], 1.0)
            v_sb.append(vt)
        for q in range(NT16):
            ps = ps_proj.tile([128, FL], F32, tag="psp")
            for c in range(NCH):
                nc.tensor.matmul(
                    ps[:], x_sb[c][:, 128 * q:128 * (q + 1)], wv_sb[c][:],
                    start=(c == 0), stop=(c == NCH - 1))
            nc.vector.tensor_copy(
                v_sb[q][:, :, 0:64],
                ps[:].rearrange("p (h d) -> p h d", h=HL))

        # ---- per head-pair: Q/K projection then attention ----
        yt_sb = [p_yt.tile([128, S], BF16, tag="yt", name=f"yt{i}") for i in range(NP)]
        qt_sb = [p_qk.tile([128, S], BF16, tag="qk", name=f"qt{i}") for i in range(NP)]
        kt_sb = [p_qk.tile([128, S], BF16, tag="qk", name=f"kt{i}") for i in range(NP)]

        for p in range(NP):
            # Q and K projections for this head pair, rope fused into drain
            for w_sb, dst in ((wq_sb, qt_sb[p]), (wk_sb, kt_sb[p])):
                for t in range(NSQ):
                    ps = ps_proj.tile([128, 512], F32, tag="psp")
                    for c in range(NCH):
                        nc.tensor.matmul(
                            ps[:],
                            w_sb[c][:, 128 * p:128 * (p + 1)],
                            x_sb[c][:, 512 * t:512 * (t + 1)],
                            start=(c == 0), stop=(c == NCH - 1))
                    nc.vector.tensor_tensor(
                        out=dst[:, 512 * t:512 * (t + 1)],
                        in0=ps[:], in1=sin_sb[:, 512 * t:512 * (t + 1)],
                        op=mult)

            # attention for heads 2p (e=0) and 2p+1 (e=1): both parities share
            # one [128,1024] S^T psum tile (cols [0:512]=e0, [512:1024]=e1) so
            # their K=64 matmuls pack PE row-groups 0-1/2-3 concurrently and
            # one exp covers both. P@V runs one chunk behind the S matmuls so
            # PE never waits on ACT.
            for t in range(NSQ):
                nchunks = 4 * (t + 1)
                o_ps = [ps_o.tile([65, 512], F32, tag="pso", name=f"o{t}_{i}") for i in range(2)]
                pv_prev = None
                for c in range(nchunks):
                    s_ps = ps_s.tile([128, 1024], F32, tag="s")
                    for e in range(2):
                        nc.tensor.matmul(
                            s_ps[:, 512 * e:512 * (e + 1)],
                            kt_sb[p][64 * e:64 * (e + 1),
                                     128 * c:128 * (c + 1)],
                            qt_sb[p][64 * e:64 * (e + 1),
                                     512 * t:512 * (t + 1)],
                            start=True, stop=True)
                    pt = p_pt.tile([128, 2, 512], BF16, tag="pt")
                    nc.scalar.activation(
                        pt[:].rearrange("p a b -> p (a b)"), s_ps[:],
                        EXP, scale=0.125)
                    cc = c - 4 * t
                    if cc >= 0:
                        # causal: keep where (col - part - 128*cc) >= 0,
                        # applied to both parity blocks at once
                        nc.gpsimd.affine_select(
                            out=pt[:], in_=pt[:],
                            compare_op=_alu().is_ge,
                            fill=0.0,
                            base=-128 * cc,
                            pattern=[[0, 2], [1, 512]],
                            channel_multiplier=-1)
                    if pv_prev is not None:
                        cp, ptp = pv_prev
                        for e in range(2):
                            nc.tensor.matmul(
                                o_ps[e][:],
                                v_sb[cp][:, 2 * p + e, :],
                                ptp[:, e, :],
                                start=(cp == 0), stop=False)
                    pv_prev = (c, pt)
                cp, ptp = pv_prev
                for e in range(2):
                    nc.tensor.matmul(
                        o_ps[e][:],
                        v_sb[cp][:, 2 * p + e, :],
                        ptp[:, e, :],
                        start=(cp == 0), stop=True)
                for e in range(2):
                    recip = p_r.tile([1, 512], F32, tag="rc")
                    nc.vector.reciprocal(recip[:], o_ps[e][64:65, :])
                    rb = p_r.tile([64, 512], F32, tag="rb")
                    nc.gpsimd.partition_broadcast(rb[:], recip[:], channels=64)
                    nc.vector.tensor_tensor(
                        out=yt_sb[p][64 * e:64 * (e + 1),
                                     512 * t:512 * (t + 1)],
                        in0=o_ps[e][0:64, :], in1=rb[:], op=mult)

        # ---- output projection: natural [sq, outD] layout ----
        for q in range(NT16):
            for o in range(2):
                ps = ps_proj.tile([128, 512], F32, tag="psp")
                for p in range(NP):
                    nc.tensor.matmul(
                        ps[:],
                        yt_sb[p][:, 128 * q:128 * (q + 1)],
                        wo_sb[p][:, 512 * o:512 * (o + 1)],
                        start=(p == 0), stop=(p == NP - 1))
                os_t = p_os.tile([128, 512], F32, tag="os")
                nc.vector.tensor_copy(os_t[:], ps[:])
                nc.sync.dma_start(
                    d_out[128 * q:128 * (q + 1), 512 * o:512 * (o + 1)],
                    os_t[:])

    nc.finalize()
    _CACHED["nc"] = nc
    return nc


def _host_prep(x, wq, wk, wv, wo):
    """Fold RoPE rotation into wq/wk; build sin table; slice per core."""
    # sin table exactly as the reference computes it (f32 throughout)
    rope_dim = AOD // 2
    j = np.arange(rope_dim, dtype=np.float32)
    thetas = (1.0 / (10000.0 ** (2.0 * j / rope_dim))).astype(np.float32)
    pos = np.arange(S, dtype=np.float32)
    angles = pos[:, None] * thetas[None, :]          # [S, 32]
    sinv = np.sin(angles).astype(np.float32)         # [S, 32]
    # sin pattern tile [128, S]: row r multiplies feature (64*pair + r%64);
    # rows r and r+32 (within each head) share sin[:, r%32]
    sin2 = np.tile(sinv.T, (4, 1)).astype(np.float32)  # [128, S]

    def fold(w):
        wr = w.reshape(D, H, 2, rope_dim)
        return np.concatenate(
            [wr[:, :, 0] - wr[:, :, 1], wr[:, :, 0] + wr[:, :, 1]],
            axis=2).reshape(D, D)

    wqf = fold(wq)
    wkf = fold(wk)

    bf = ml_dtypes.bfloat16
    in_maps = []
    for c in range(NCORES):
        b, g = divmod(c, 2)
        sl = slice(g * FL, (g + 1) * FL)
        in_maps.append({
            "xT": np.ascontiguousarray(x[b].T).astype(bf),
            "wq": np.ascontiguousarray(wqf[:, sl]).astype(bf),
            "wk": np.ascontiguousarray(wkf[:, sl]).astype(bf),
            "wv": np.ascontiguousarray(wv[:, sl]).astype(bf),
            "wo": np.ascontiguousarray(wo[sl, :]).astype(bf),
            "sin": sin2,
        })
    return in_maps


def kernel(x, wq, wk, wv, wo, bo):
    x = np.asarray(x, dtype=np.float32)
    nc = build_nc()
    in_maps = _host_prep(np.asarray(x, np.float32), np.asarray(wq, np.float32),
                         np.asarray(wk, np.float32), np.asarray(wv, np.float32),
                         np.asarray(wo, np.float32))
    res = run_bass_kernel_spmd(nc, in_maps, list(range(NCORES)))
    out = np.empty((B, S, D), np.float32)
    bo32 = np.asarray(bo, np.float32)
    for b in range(B):
        out[b] = res.results[2 * b]["out"] + res.results[2 * b + 1]["out"] + bo32
    return out


# revision 14
# speedup vs baseline: 56.9881x; 56.9881x over previous
"""Causal multi-head attention with (buggy-faithful) RoPE on 8 trn2 cores.

Problem: B=4, S=2048, D=1024, H=16 heads of dim 64, fp32.
Sharding: core c handles batch b=c//2 and head-group g=c%2 (8 heads).
Each core computes partial_out = attn(x_b, heads g) @ wo[rows g]; the host
sums the two partials per batch and adds the bias.

Key host-side preprocessing:
- The reference's RoPE (with its cos-overwritten-by-sin bug) reduces to
  q' = (q @ (I + R')) * sin_pattern, where R' swaps/negates half-dims.
  (I + R') is folded into wq/wk on the host, so on-device RoPE is a single
  elementwise multiply fused into the PSUM->SBUF drain of the projections.
- x is supplied transposed (xT [D, S]) so every matmul contraction dim lands
  on SBUF partitions naturally.

On-device layout (per core):
- QT/KT [512, 2048] feature-major (head pairs stacked per 128-partition chunk)
- V [2048, 520] sequence-major, 65 cols per head (64 + ones col -> softmax
  denominator accumulates for free in the P@V matmul)
- S^T [sk, sq] per head: softmax denom is a partition-dim sum, handled by the
  ones column; exp needs no max subtraction (|scores/8| < ~3).
"""

import numpy as np
import ml_dtypes

import concourse.bacc as bacc
import concourse.mybir as mybir
import concourse.tile as tile
from concourse.bass_utils import run_bass_kernel_spmd

B, S, D = 4, 2048, 1024
H = 16
AOD = 64
HL = 8            # heads per core
FL = HL * AOD     # 512 local features
NCORES = 8
NCH = D // 128    # 8 contraction chunks for projections
NSQ = 4           # sq tiles of 512
NP = FL // 128    # 4 feature chunks (head pairs)
NT16 = S // 128   # 16 seq chunks of 128

F32 = mybir.dt.float32
BF16 = mybir.dt.bfloat16
EXP = mybir.ActivationFunctionType.Exp

_CACHED = {}


def _alu():
    from concourse.alu_op_type import AluOpType
    return AluOpType


def _emit_body(nc, P, dram, rep):
    """One full forward pass for this core's shard."""
    mult = _alu().mult
    is_ge = _alu().is_ge
    d_xT, d_wq, d_wk, d_wv, d_wo, d_sin, d_out = dram
    (p_x, p_w, p_wo, p_qk, p_v, p_sin, p_pt, p_yt, p_r, p_os,
     ps_proj, ps_s, ps_o) = P
    R = f"r{rep}"

    # ---- resident loads ----
    sin_sb = p_sin.tile([128, S], F32, tag="sin", name=f"{R}sin_sb")
    nc.sync.dma_start(sin_sb[:], d_sin)
    x_sb = []
    for c in range(NCH):
        t = p_x.tile([128, S], BF16, tag="x", name=f"{R}x{c}")
        nc.sync.dma_start(t[:], d_xT[128 * c:128 * (c + 1), :])
        x_sb.append(t)
    wq_sb, wk_sb, wv_sb = [], [], []
    for lst, drm, nm in ((wq_sb, d_wq, "wq"), (wk_sb, d_wk, "wk"),
                         (wv_sb, d_wv, "wv")):
        for c in range(NCH):
            t = p_w.tile([128, FL], BF16, tag="w", name=f"{R}{nm}{c}")
            nc.sync.dma_start(t[:], drm[128 * c:128 * (c + 1), :])
            lst.append(t)
    wo_sb = []
    for p in range(NP):
        t = p_wo.tile([128, D], BF16, tag="wo", name=f"{R}wo{p}")
        nc.sync.dma_start(t[:], d_wo[128 * p:128 * (p + 1), :])
        wo_sb.append(t)

    # ---- V projection: V[sq, f] natural layout, 65-wide per head ----
    v_sb = []
    for q in range(NT16):
        vt = p_v.tile([128, HL, 65], BF16, tag="v", name=f"{R}v{q}")
        nc.gpsimd.memset(vt[:, :, 64:65], 1.0)
        v_sb.append(vt)
    for q in range(NT16):
        ps = ps_proj.tile([128, FL], F32, tag="psp", name=f"{R}vps{q}")
        for c in range(NCH):
            nc.tensor.matmul(
                ps[:], x_sb[c][:, 128 * q:128 * (q + 1)], wv_sb[c][:],
                start=(c == 0), stop=(c == NCH - 1))
        nc.vector.tensor_copy(
            v_sb[q][:, :, 0:64],
            ps[:].rearrange("p (h d) -> p h d", h=HL))

    # ---- per head-pair: Q/K projection then attention ----
    yt_sb = [p_yt.tile([128, S], BF16, tag="yt", name=f"{R}yt{i}")
             for i in range(NP)]
    qt_sb = [p_qk.tile([128, S], BF16, tag="qk", name=f"{R}qt{i}")
             for i in range(NP)]
    kt_sb = [p_qk.tile([128, S], BF16, tag="qk", name=f"{R}kt{i}")
             for i in range(NP)]

    for p in range(NP):
        # Q and K projections for this head pair, rope fused into the drain
        for w_sb, dst in ((wq_sb, qt_sb[p]), (wk_sb, kt_sb[p])):
            for t in range(NSQ):
                ps = ps_proj.tile([128, 512], F32, tag="psp",
                                  name=f"{R}qkps{p}{t}")
                for c in range(NCH):
                    nc.tensor.matmul(
                        ps[:],
                        w_sb[c][:, 128 * p:128 * (p + 1)],
                        x_sb[c][:, 512 * t:512 * (t + 1)],
                        start=(c == 0), stop=(c == NCH - 1))
                nc.vector.tensor_tensor(
                    out=dst[:, 512 * t:512 * (t + 1)],
                    in0=ps[:], in1=sin_sb[:, 512 * t:512 * (t + 1)],
                    op=mult)

        # attention for heads 2p (e=0) and 2p+1 (e=1): both parities share
        # one [128,1024] S^T psum tile (cols [0:512]=e0, [512:1024]=e1) so
        # their K=64 matmuls pack PE row-groups 0-1/2-3 concurrently and one
        # exp covers both. P@V runs one chunk behind the S matmuls so PE
        # never waits on ACT.
        for t in range(NSQ):
            nchunks = 4 * (t + 1)
            o_ps = [ps_o.tile([65, 512], F32, tag="pso",
                              name=f"{R}o{p}_{t}_{i}") for i in range(2)]
            pv_prev = None
            for c in range(nchunks):
                s_ps = ps_s.tile([128, 1024], F32, tag="s",
                                 name=f"{R}s{p}_{t}_{c}")
                for e in range(2):
                    nc.tensor.matmul(
                        s_ps[:, 512 * e:512 * (e + 1)],
                        kt_sb[p][64 * e:64 * (e + 1), 128 * c:128 * (c + 1)],
                        qt_sb[p][64 * e:64 * (e + 1), 512 * t:512 * (t + 1)],
                        start=True, stop=True)
                pt = p_pt.tile([128, 2, 512], BF16, tag="pt",
                               name=f"{R}pt{p}_{t}_{c}")
                nc.scalar.activation(
                    pt[:].rearrange("p a b -> p (a b)"), s_ps[:],
                    EXP, scale=0.125)
                cc = c - 4 * t
                if cc >= 0:
                    # causal: keep where (col - part - 128*cc) >= 0, both
                    # parity blocks at once
                    nc.gpsimd.affine_select(
                        out=pt[:], in_=pt[:],
                        compare_op=is_ge,
                        fill=0.0,
                        base=-128 * cc,
                        pattern=[[0, 2], [1, 512]],
                        channel_multiplier=-1)
                if pv_prev is not None:
                    cp, ptp = pv_prev
                    for e in range(2):
                        nc.tensor.matmul(
                            o_ps[e][:], v_sb[cp][:, 2 * p + e, :],
                            ptp[:, e, :],
                            start=(cp == 0), stop=False)
                pv_prev = (c, pt)
            cp, ptp = pv_prev
            for e in range(2):
                nc.tensor.matmul(
                    o_ps[e][:], v_sb[cp][:, 2 * p + e, :], ptp[:, e, :],
                    start=(cp == 0), stop=True)
            for e in range(2):
                recip = p_r.tile([1, 512], F32, tag="rc",
                                 name=f"{R}rc{p}_{t}{e}")
                nc.vector.reciprocal(recip[:], o_ps[e][64:65, :])
                rb = p_r.tile([64, 512], F32, tag="rb", name=f"{R}rb{p}_{t}{e}")
                nc.gpsimd.partition_broadcast(rb[:], recip[:], channels=64)
                nc.vector.tensor_tensor(
                    out=yt_sb[p][64 * e:64 * (e + 1), 512 * t:512 * (t + 1)],
                    in0=o_ps[e][0:64, :], in1=rb[:], op=mult)

    # ---- output projection: natural [sq, outD] layout ----
    for q in range(NT16):
        for o in range(2):
            ps = ps_proj.tile([128, 512], F32, tag="psp", name=f"{R}ops{q}{o}")
            for p in range(NP):
                nc.tensor.matmul(
                    ps[:],
                    yt_sb[p][:, 128 * q:128 * (q + 1)],
                    wo_sb[p][:, 512 * o:512 * (o + 1)],
                    start=(p == 0), stop=(p == NP - 1))
            os_t = p_os.tile([128, 512], F32, tag="os", name=f"{R}os{q}{o}")
            nc.vector.tensor_copy(os_t[:], ps[:])
            nc.sync.dma_start(
                d_out[128 * q:128 * (q + 1), 512 * o:512 * (o + 1)], os_t[:])


def build_nc(reps=1):
    key = ("nc", reps)
    if key in _CACHED:
        return _CACHED[key]
    from contextlib import ExitStack

    nc = bacc.Bacc("TRN2", target_bir_lowering=False, debug=False,
                   num_devices=NCORES)
    dram = (
        nc.dram_tensor("xT", [D, S], BF16, kind="ExternalInput").ap(),
        nc.dram_tensor("wq", [D, FL], BF16, kind="ExternalInput").ap(),
        nc.dram_tensor("wk", [D, FL], BF16, kind="ExternalInput").ap(),
        nc.dram_tensor("wv", [D, FL], BF16, kind="ExternalInput").ap(),
        nc.dram_tensor("wo", [FL, D], BF16, kind="ExternalInput").ap(),
        nc.dram_tensor("sin", [128, S], F32, kind="ExternalInput").ap(),
        nc.dram_tensor("out", [S, D], F32, kind="ExternalOutput").ap(),
    )

    import os
    trace_sim = bool(os.environ.get("KTRACE"))
    with tile.TileContext(nc, trace_sim=trace_sim) as tc, ExitStack() as ctx:
        P = (
            ctx.enter_context(tc.tile_pool(name="x", bufs=NCH)),
            ctx.enter_context(tc.tile_pool(name="w", bufs=3 * NCH)),
            ctx.enter_context(tc.tile_pool(name="wo", bufs=NP)),
            ctx.enter_context(tc.tile_pool(name="qk", bufs=2 * NP)),
            ctx.enter_context(tc.tile_pool(name="v", bufs=NT16)),
            ctx.enter_context(tc.tile_pool(name="sin", bufs=1)),
            ctx.enter_context(tc.tile_pool(name="pt", bufs=4)),
            ctx.enter_context(tc.tile_pool(name="yt", bufs=NP)),
            ctx.enter_context(tc.tile_pool(name="r", bufs=4)),
            ctx.enter_context(tc.tile_pool(name="os", bufs=4)),
            ctx.enter_context(tc.tile_pool(name="psp", bufs=2, space="PSUM")),
            ctx.enter_context(tc.tile_pool(name="pss", bufs=2, space="PSUM")),
            ctx.enter_context(tc.tile_pool(name="pso", bufs=2, space="PSUM")),
        )
        for rep in range(reps):
            _emit_body(nc, P, dram, rep)

    nc.finalize()
    _CACHED[key] = nc
    return nc


def _host_prep(x, wq, wk, wv, wo):
    """Fold RoPE rotation into wq/wk; build sin table; slice per core."""
    # sin table exactly as the reference computes it (f32 throughout)
    rope_dim = AOD // 2
    j = np.arange(rope_dim, dtype=np.float32)
    thetas = (1.0 / (10000.0 ** (2.0 * j / rope_dim))).astype(np.float32)
    pos = np.arange(S, dtype=np.float32)
    angles = pos[:, None] * thetas[None, :]          # [S, 32]
    sinv = np.sin(angles).astype(np.float32)         # [S, 32]
    # sin pattern tile [128, S]: row r multiplies feature (64*pair + r%64);
    # rows r and r+32 (within each head) share sin[:, r%32]
    sin2 = np.tile(sinv.T, (4, 1)).astype(np.float32)  # [128, S]

    def fold(w):
        wr = w.reshape(D, H, 2, rope_dim)
        return np.concatenate(
            [wr[:, :, 0] - wr[:, :, 1], wr[:, :, 0] + wr[:, :, 1]],
            axis=2).reshape(D, D)

    wqf = fold(wq)
    wkf = fold(wk)

    bf = ml_dtypes.bfloat16
    in_maps = []
    for c in range(NCORES):
        b, g = divmod(c, 2)
        sl = slice(g * FL, (g + 1) * FL)
        in_maps.append({
            "xT": np.ascontiguousarray(x[b].T).astype(bf),
            "wq": np.ascontiguousarray(wqf[:, sl]).astype(bf),
            "wk": np.ascontiguousarray(wkf[:, sl]).astype(bf),
            "wv": np.ascontiguousarray(wv[:, sl]).astype(bf),
            "wo": np.ascontiguousarray(wo[sl, :]).astype(bf),
            "sin": sin2,
        })
    return in_maps


def kernel(x, wq, wk, wv, wo, bo):
    nc = build_nc()
    in_maps = _host_prep(np.asarray(x, np.float32), np.asarray(wq, np.float32),
                         np.asarray(wk, np.float32), np.asarray(wv, np.float32),
                         np.asarray(wo, np.float32))
    res = run_bass_kernel_spmd(nc, in_maps, list(range(NCORES)))
    out = np.empty((B, S, D), np.float32)
    bo32 = np.asarray(bo, np.float32)
    for b in range(B):
        out[b] = res.results[2 * b]["out"] + res.results[2 * b + 1]["out"] + bo32
    return out
